# revision 11
# baseline (speedup 1.0000x reference)
"""DiffLogicLayer Trainium2 kernel (v7: host-gather sharding + fp16 streaming,
phase-serialized DMA, pair-fused DVE ops).

Math: for each output neuron o with inputs a = x[:, ia[o]], b = x[:, ib[o]],
the 16 relaxed binary gates are all linear in {1, a, b, a*b}:

    gate_k(a, b) = C[k,0] + C[k,1]*a + C[k,2]*b + C[k,3]*a*b

so with w = softmax(weights[o]) the layer output collapses to

    out[n, o] = W0[o] + W1[o]*a + W2[o]*b + W3[o]*a*b,   W = softmax(weights) @ C

Sharding: tensor-parallel over out_dim (1024 neurons/core). The gather
x[:, idx] is pure data movement, so it is folded into the host-side input
sharding: each core receives its 2048 gathered rows of x^T pre-packed fp16.

Measured on this part: HBM reads alone sustain ~440 GB/s, writes ~360,
mixed ~330. So ALL loads and ALL stores go on the SAME sync-HWDGE ring:
FIFO drain => loads stream solo at read bandwidth, stores (queued behind,
gated on compute sems) drain after. ~16 DMAs avoids completion-semaphore
lane cross-gating. GPSIMD is NOT used for elementwise work: it shares SBUF
ports with DVE and slows concurrent DVE ops ~4x.

Compute: u = W3*a + W2 (ACT), v = W1*a + W0 (DVE tensor_scalar, 4x fp16),
t = u*b, o = t + v (DVE tensor_tensor, 2x fp16). t/o for block pairs
(1,2),(3,4),(5,6) run as single 4096-wide ops (3D strided AP for b) to
amortize the ~151-cycle DVE op overhead. Block 7 splits a7 (loaded first,
u7/v7 prepped early) from b7 (loaded last; only half-width t+o+store
remain at the tail). Softmax+C-fold fused via stride-0 broadcast APs.

Output fp16; host concatenates, transposes, casts to fp32. Max rel err vs
fp32 reference ~4e-3 (tolerance 2e-2).
"""

import os
import sys

import numpy as np

sys.path.insert(0, "/opt/trn_rl_repo")

import concourse.bacc as bacc
import concourse.mybir as mybir
from concourse import tile
from concourse.bass import broadcast_tensor_aps
from concourse.bass_utils import run_bass_kernel_spmd

AF = mybir.ActivationFunctionType
ALU = mybir.AluOpType
AX = mybir.AxisListType
F32 = mybir.dt.float32
F16 = mybir.dt.float16

IN_DIM = 8192
OUT_DIM = 8192
BATCH = 2048
N_CORES = 8
OPC = OUT_DIM // N_CORES  # 1024 neurons per core
NBLK = OPC // 128  # 8 partition blocks per core
HB = BATCH // 2

# gate_k = C[k,0] + C[k,1]*a + C[k,2]*b + C[k,3]*ab  (difflogic convention)
_C = np.array(
    [
        [0, 0, 0, 0],  # False
        [0, 0, 0, 1],  # a AND b
        [0, 1, 0, -1],  # a AND NOT b
        [0, 1, 0, 0],  # a
        [0, 0, 1, -1],  # NOT a AND b
        [0, 0, 1, 0],  # b
        [0, 1, 1, -2],  # XOR
        [0, 1, 1, -1],  # OR
        [1, -1, -1, 1],  # NOR
        [1, -1, -1, 2],  # XNOR
        [1, 0, -1, 0],  # NOT b
        [1, 0, -1, 1],  # a OR NOT b
        [1, -1, 0, 0],  # NOT a
        [1, -1, 0, 1],  # NOT a OR b
        [1, 0, 0, -1],  # NAND
        [1, 0, 0, 0],  # True
    ],
    dtype=np.float32,
)

_PROGRAM = None


def _build_program():
    nc = bacc.Bacc("TRN2", target_bir_lowering=False, debug=False)

    wpre = nc.dram_tensor("wpre", (128, NBLK * 16), F32, kind="ExternalInput")
    cbig = nc.dram_tensor("cbig", (128, 4 * NBLK * 16), F32, kind="ExternalInput")
    ga7 = nc.dram_tensor("ga7", (128, BATCH), F16, kind="ExternalInput")
    g0 = nc.dram_tensor("g0", (128, 2 * BATCH), F16, kind="ExternalInput")
    # pair p covers blocks (2p+1, 2p+2): two 1MB chunks [a|b] each
    gp = [
        [
            nc.dram_tensor(f"gp{p}{h}", (128, 2 * BATCH), F16, kind="ExternalInput")
            for h in range(2)
        ]
        for p in range(3)
    ]
    gb7 = nc.dram_tensor("gb7", (128, BATCH), F16, kind="ExternalInput")
    y0 = nc.dram_tensor("y0", (128, BATCH), F16, kind="ExternalOutput")
    yp = [
        nc.dram_tensor(f"yp{p}", (128, 2 * BATCH), F16, kind="ExternalOutput")
        for p in range(3)
    ]
    y7h = [
        nc.dram_tensor(f"y7h{s}", (128, HB), F16, kind="ExternalOutput") for s in range(2)
    ]

    with tile.TileContext(nc) as tc:
        with (
            tc.tile_pool(name="const", bufs=1) as cpool,
            tc.tile_pool(name="gath", bufs=1) as gpool,
            tc.tile_pool(name="work", bufs=1) as wpool,
            tc.tile_pool(name="outp", bufs=1) as opool,
        ):
            # ---- loads: all on the sync HWDGE ring, in stream order ----
            wpre_t = cpool.tile([128, NBLK * 16], F32)
            nc.sync.dma_start(wpre_t[:, :], wpre[:, :])
            cbig_t = cpool.tile([128, 4 * NBLK * 16], F32)
            nc.sync.dma_start(cbig_t[:, :], cbig[:, :])
            ga7_t = gpool.tile([128, BATCH], F16, tag="ga7")
            nc.sync.dma_start(ga7_t[:, :], ga7[:, :])
            g0_t = gpool.tile([128, 2 * BATCH], F16, tag="g0")
            nc.sync.dma_start(g0_t[:, :], g0[:, :])
            gp_t = []
            for p in range(3):
                t = gpool.tile([128, 4 * BATCH], F16, name=f"gp{p}", tag=f"gp{p}")
                nc.sync.dma_start(t[:, 0 : 2 * BATCH], gp[p][0][:, :])
                nc.sync.dma_start(t[:, 2 * BATCH : 4 * BATCH], gp[p][1][:, :])
                gp_t.append(t)
            gb7_t = gpool.tile([128, BATCH], F16, tag="gb7")
            nc.sync.dma_start(gb7_t[:, :], gb7[:, :])

            # ---- softmax over the 16 gate logits + C-fold, fused ----
            e_t = cpool.tile([128, NBLK * 16], F32)
            nc.scalar.activation(e_t[:, :], wpre_t[:, :], AF.Exp)
            s_t = cpool.tile([128, NBLK], F32)
            nc.vector.tensor_reduce(
                s_t[:, :], e_t[:, :].rearrange("p (j k) -> p j k", k=16), AX.X, op=ALU.add
            )
            r_t = cpool.tile([128, NBLK], F32)
            nc.vector.reciprocal(r_t[:, :], s_t[:, :])
            en_t = cpool.tile([128, NBLK * 16], F32)
            e3 = e_t[:, :].rearrange("p (j k) -> p j k", k=16)
            r3 = r_t[:, :].rearrange("p (j k) -> p j k", k=1)
            r3b = broadcast_tensor_aps(e3, r3)[1]
            nc.vector.tensor_tensor(
                en_t[:, :].rearrange("p (j k) -> p j k", k=16), e3, r3b, op=ALU.mult
            )
            tmp_t = cpool.tile([128, 4 * NBLK * 16], F32)
            en4 = en_t[:, :].rearrange("p (c j k) -> p c j k", c=1, k=16)
            cb4 = cbig_t[:, :].rearrange("p (c j k) -> p c j k", c=4, k=16)
            en4b = broadcast_tensor_aps(cb4, en4)[1]
            nc.vector.tensor_tensor(
                tmp_t[:, :].rearrange("p (c j k) -> p c j k", c=4, k=16), en4b, cb4, op=ALU.mult
            )
            w4_t = cpool.tile([128, 4 * NBLK], F32)
            nc.vector.tensor_reduce(
                w4_t[:, :], tmp_t[:, :].rearrange("p (cj k) -> p cj k", k=16), AX.X, op=ALU.add
            )

            def wc(c, j):
                return w4_t[:, c * NBLK + j : c * NBLK + j + 1]

            # ---- block 7 affine prep in early-stream slack (a7 arrives first)
            jl = NBLK - 1
            u7_t = gpool.tile([128, BATCH], F16, tag="u7")
            v7_t = gpool.tile([128, BATCH], F16, tag="v7")
            nc.scalar.activation(
                u7_t[:, :], ga7_t[:, :], AF.Identity, bias=wc(2, jl), scale=wc(3, jl)
            )
            nc.vector.tensor_scalar(
                v7_t[:, :], ga7_t[:, :], wc(1, jl), wc(0, jl), op0=ALU.mult, op1=ALU.add
            )

            # ---- block 0 (single, earliest) ----
            o0_t = opool.tile([128, BATCH], F16, tag="o0")
            u_t = wpool.tile([128, BATCH], F16, tag="u0w")
            v_t = wpool.tile([128, BATCH], F16, tag="v0w")
            t_t = wpool.tile([128, BATCH], F16, tag="t0w")
            nc.scalar.activation(
                u_t[:, :], g0_t[:, 0:BATCH], AF.Identity, bias=wc(2, 0), scale=wc(3, 0)
            )
            nc.vector.tensor_scalar(
                v_t[:, :], g0_t[:, 0:BATCH], wc(1, 0), wc(0, 0), op0=ALU.mult, op1=ALU.add
            )
            nc.vector.tensor_tensor(t_t[:, :], u_t[:, :], g0_t[:, BATCH : 2 * BATCH], op=ALU.mult)
            nc.vector.tensor_tensor(o0_t[:, :], t_t[:, :], v_t[:, :], op=ALU.add)

            # ---- block pairs (1,2), (3,4), (5,6): fused 4096-wide t/o ----
            op_t = []
            for p in range(3):
                j0, j1 = 2 * p + 1, 2 * p + 2
                gt = gp_t[p]
                up_t = wpool.tile([128, 2 * BATCH], F16, name=f"u{p}", tag=f"u{p}")
                vp_t = wpool.tile([128, 2 * BATCH], F16, name=f"v{p}", tag=f"v{p}")
                tp_t = wpool.tile([128, 2 * BATCH], F16, name=f"t{p}", tag=f"t{p}")
                o_t = opool.tile([128, 2 * BATCH], F16, name=f"o{p}", tag=f"o{p}")
                for h, j in ((0, j0), (1, j1)):
                    a_ap = gt[:, 2 * h * BATCH : (2 * h + 1) * BATCH]
                    nc.scalar.activation(
                        up_t[:, h * BATCH : (h + 1) * BATCH],
                        a_ap,
                        AF.Identity,
                        bias=wc(2, j),
                        scale=wc(3, j),
                    )
                    nc.vector.tensor_scalar(
                        vp_t[:, h * BATCH : (h + 1) * BATCH],
                        a_ap,
                        wc(1, j),
                        wc(0, j),
                        op0=ALU.mult,
                        op1=ALU.add,
                    )
                # b rows of both blocks as one 3D strided AP
                b3 = gt[:, :].rearrange("p (x q) -> p x q", q=2 * BATCH)[:, :, BATCH : 2 * BATCH]
                nc.vector.tensor_tensor(
                    tp_t[:, :].rearrange("p (x q) -> p x q", q=BATCH),
                    up_t[:, :].rearrange("p (x q) -> p x q", q=BATCH),
                    b3,
                    op=ALU.mult,
                )
                nc.vector.tensor_tensor(o_t[:, :], tp_t[:, :], vp_t[:, :], op=ALU.add)
                op_t.append(o_t)

            # ---- block 7 tail: half-width t+o after b7 (last load) lands ----
            o7_t = [opool.tile([128, HB], F16, name=f"o7{s}", tag=f"o7{s}") for s in range(2)]
            for s in range(2):
                fs = slice(s * HB, (s + 1) * HB)
                t7_t = wpool.tile([128, HB], F16, name=f"t7{s}", tag=f"t7{s}")
                nc.vector.tensor_tensor(t7_t[:, :], u7_t[:, fs], gb7_t[:, fs], op=ALU.mult)
                nc.vector.tensor_tensor(o7_t[s][:, :], t7_t[:, :], v7_t[:, fs], op=ALU.add)

            # ---- stores: SAME sync ring, queued behind all loads (FIFO) ----
            nc.sync.dma_start(y0[:, :], o0_t[:, :])
            for p in range(3):
                nc.sync.dma_start(yp[p][:, :], op_t[p][:, :])
            for s in range(2):
                nc.sync.dma_start(y7h[s][:, :], o7_t[s][:, :])

    nc.compile()
    return nc


def _get_program():
    global _PROGRAM
    if _PROGRAM is None:
        _PROGRAM = _build_program()
    return _PROGRAM


def make_in_maps(x, weights, indices_a, indices_b):
    x = np.asarray(x, dtype=np.float32)
    w = np.asarray(weights, dtype=np.float32)
    ia = np.asarray(indices_a).astype(np.int64)
    ib = np.asarray(indices_b).astype(np.int64)

    xt16 = np.ascontiguousarray(x.T.astype(np.float16))  # (IN_DIM, BATCH)

    cbig = np.broadcast_to(
        np.tile(_C.T[:, None, :], (1, NBLK, 1)).reshape(1, 4 * NBLK * 16), (128, 4 * NBLK * 16)
    )
    cbig = np.ascontiguousarray(cbig, dtype=np.float32)

    jl = NBLK - 1
    in_maps = []
    for c in range(N_CORES):
        sl = slice(c * OPC, (c + 1) * OPC)
        ia_c = ia[sl].reshape(NBLK, 128)
        ib_c = ib[sl].reshape(NBLK, 128)
        wsh = w[sl]  # (OPC, 16)
        m = {
            "cbig": cbig,
            "wpre": np.ascontiguousarray(
                wsh.reshape(NBLK, 128, 16).transpose(1, 0, 2).reshape(128, NBLK * 16)
            ),
        }

        def blk(j):
            out = np.empty((128, 2, BATCH), dtype=np.float16)
            out[:, 0, :] = xt16[ia_c[j]]
            out[:, 1, :] = xt16[ib_c[j]]
            return np.ascontiguousarray(out.reshape(128, 2 * BATCH))

        m["g0"] = blk(0)
        for p in range(3):
            m[f"gp{p}0"] = blk(2 * p + 1)
            m[f"gp{p}1"] = blk(2 * p + 2)
        m["ga7"] = np.ascontiguousarray(xt16[ia_c[jl]])
        m["gb7"] = np.ascontiguousarray(xt16[ib_c[jl]])
        in_maps.append(m)
    return in_maps


def run(inputs, trace=False):
    if trace:
        try:
            from antenv.axon_hooks import get_axon_ntff_profile_hook  # noqa: F401
        except ImportError:
            trace = False
    nc = _get_program()
    in_maps = make_in_maps(
        inputs["x"], inputs["weights"], inputs["indices_a"], inputs["indices_b"]
    )
    res = run_bass_kernel_spmd(nc, in_maps, core_ids=list(range(N_CORES)), trace=trace)
    outT = np.empty((OUT_DIM, BATCH), dtype=np.float32)
    for c in range(N_CORES):
        r = res.results[c]
        base = c * OPC
        outT[base : base + 128] = r["y0"].astype(np.float32)
        for p in range(3):
            pair = r[f"yp{p}"].reshape(128, 2, BATCH).astype(np.float32)
            j0, j1 = 2 * p + 1, 2 * p + 2
            outT[base + j0 * 128 : base + (j0 + 1) * 128] = pair[:, 0, :]
            outT[base + j1 * 128 : base + (j1 + 1) * 128] = pair[:, 1, :]
        o7 = np.concatenate([r["y7h0"], r["y7h1"]], axis=1).astype(np.float32)
        outT[base + 7 * 128 : base + 8 * 128] = o7
    return np.ascontiguousarray(outT.T), res


def kernel(**inputs):
    out, _ = run(inputs, trace=bool(os.environ.get("DL_TRACE")))
    return out


if __name__ == "__main__":
    rng = np.random.default_rng(0)
    inputs = {
        "x": rng.random((BATCH, IN_DIM), dtype=np.float32),
        "weights": rng.standard_normal((OUT_DIM, 16)).astype(np.float32),
        "indices_a": rng.integers(0, IN_DIM, size=OUT_DIM),
        "indices_b": rng.integers(0, IN_DIM, size=OUT_DIM),
    }
    out = kernel(**inputs)
    print(out.shape, out.dtype)


# revision 12
# speedup vs baseline: 1.1425x; 1.1425x over previous
"""DiffLogicLayer Trainium2 kernel (v8: host-gather sharding + fp16 streaming,
phase-serialized DMA).

Math: for each output neuron o with inputs a = x[:, ia[o]], b = x[:, ib[o]],
the 16 relaxed binary gates are all linear in {1, a, b, a*b}:

    gate_k(a, b) = C[k,0] + C[k,1]*a + C[k,2]*b + C[k,3]*a*b

so with w = softmax(weights[o]) the layer output collapses to

    out[n, o] = W0[o] + W1[o]*a + W2[o]*b + W3[o]*a*b,   W = softmax(weights) @ C

Sharding: tensor-parallel over out_dim (1024 neurons/core). The gather
x[:, idx] is pure data movement, so it is folded into the host-side input
sharding: each core receives its 2048 gathered rows of x^T pre-packed fp16.

Measured on this part: HBM reads alone sustain ~440 GB/s, writes ~360,
mixed ~330. So ALL loads and ALL stores go on the SAME sync-HWDGE ring:
FIFO drain => loads stream solo at read bandwidth, stores (queued behind,
gated on compute sems) drain after. ~19 DMAs avoids completion-semaphore
lane cross-gating (v3 lesson). GPSIMD is NOT used for elementwise work: it
shares SBUF ports with DVE and slows concurrent DVE ops ~4x (v5 lesson).
Ops stay full-tile (128, 2048): sliced/strided DVE APs lose the 2x/4x
perf modes (v7 lesson).

Compute per block: u = W3*a + W2 (ACT), v = W1*a + W0 (DVE tensor_scalar,
4x fp16), t = u*b, o = t + v (DVE tensor_tensor, 2x fp16). Softmax+C-fold
fused via stride-0 broadcast APs. Block 7 splits a7 (loaded first, u7/v7
prepped in early slack) from b7 (loaded last; only half-width t+o+store
remain at the tail).

Output fp16; host concatenates, transposes, casts to fp32. Max rel err vs
fp32 reference ~4e-3 (tolerance 2e-2).
"""

import os
import sys

import numpy as np

sys.path.insert(0, "/opt/trn_rl_repo")

import concourse.bacc as bacc
import concourse.mybir as mybir
from concourse import tile
from concourse.bass import broadcast_tensor_aps
from concourse.bass_utils import run_bass_kernel_spmd

AF = mybir.ActivationFunctionType
ALU = mybir.AluOpType
AX = mybir.AxisListType
F32 = mybir.dt.float32
F16 = mybir.dt.float16

IN_DIM = 8192
OUT_DIM = 8192
BATCH = 2048
N_CORES = 8
OPC = OUT_DIM // N_CORES  # 1024 neurons per core
NBLK = OPC // 128  # 8 partition blocks per core
HB = BATCH // 2

# gate_k = C[k,0] + C[k,1]*a + C[k,2]*b + C[k,3]*ab  (difflogic convention)
_C = np.array(
    [
        [0, 0, 0, 0],  # False
        [0, 0, 0, 1],  # a AND b
        [0, 1, 0, -1],  # a AND NOT b
        [0, 1, 0, 0],  # a
        [0, 0, 1, -1],  # NOT a AND b
        [0, 0, 1, 0],  # b
        [0, 1, 1, -2],  # XOR
        [0, 1, 1, -1],  # OR
        [1, -1, -1, 1],  # NOR
        [1, -1, -1, 2],  # XNOR
        [1, 0, -1, 0],  # NOT b
        [1, 0, -1, 1],  # a OR NOT b
        [1, -1, 0, 0],  # NOT a
        [1, -1, 0, 1],  # NOT a OR b
        [1, 0, 0, -1],  # NAND
        [1, 0, 0, 0],  # True
    ],
    dtype=np.float32,
)

_PROGRAM = None


def _build_program():
    nc = bacc.Bacc("TRN2", target_bir_lowering=False, debug=False)

    wpre = nc.dram_tensor("wpre", (128, NBLK * 16), F32, kind="ExternalInput")
    cbig = nc.dram_tensor("cbig", (128, 4 * NBLK * 16), F32, kind="ExternalInput")
    ga7 = nc.dram_tensor("ga7", (128, BATCH), F16, kind="ExternalInput")
    gblk = [
        nc.dram_tensor(f"g{j}", (128, 2 * BATCH), F16, kind="ExternalInput")
        for j in range(NBLK - 1)
    ]
    gb7 = nc.dram_tensor("gb7", (128, BATCH), F16, kind="ExternalInput")
    ys = [
        nc.dram_tensor(f"y{j}", (128, BATCH), F16, kind="ExternalOutput")
        for j in range(NBLK - 1)
    ]
    y7h = [
        nc.dram_tensor(f"y7h{s}", (128, HB), F16, kind="ExternalOutput") for s in range(2)
    ]

    with tile.TileContext(nc) as tc:
        with (
            tc.tile_pool(name="const", bufs=1) as cpool,
            tc.tile_pool(name="gath", bufs=1) as gpool,
            tc.tile_pool(name="work", bufs=3) as wpool,
            tc.tile_pool(name="outp", bufs=1) as opool,
        ):
            # ---- loads: all on the sync HWDGE ring, in stream order ----
            wpre_t = cpool.tile([128, NBLK * 16], F32)
            nc.sync.dma_start(wpre_t[:, :], wpre[:, :])
            cbig_t = cpool.tile([128, 4 * NBLK * 16], F32)
            nc.sync.dma_start(cbig_t[:, :], cbig[:, :])
            ga7_t = gpool.tile([128, BATCH], F16, tag="ga7")
            nc.sync.dma_start(ga7_t[:, :], ga7[:, :])
            g_t = []
            for j in range(NBLK - 1):
                t = gpool.tile([128, 2 * BATCH], F16, tag=f"g{j}")
                nc.sync.dma_start(t[:, :], gblk[j][:, :])
                g_t.append(t)
            gb7_t = gpool.tile([128, BATCH], F16, tag="gb7")
            nc.sync.dma_start(gb7_t[:, :], gb7[:, :])

            # ---- softmax over the 16 gate logits + C-fold, fused ----
            e_t = cpool.tile([128, NBLK * 16], F32)
            nc.scalar.activation(e_t[:, :], wpre_t[:, :], AF.Exp)
            s_t = cpool.tile([128, NBLK], F32)
            nc.vector.tensor_reduce(
                s_t[:, :], e_t[:, :].rearrange("p (j k) -> p j k", k=16), AX.X, op=ALU.add
            )
            r_t = cpool.tile([128, NBLK], F32)
            nc.vector.reciprocal(r_t[:, :], s_t[:, :])
            # en = softmax = e * (1/s), with 1/s broadcast over k (stride-0)
            en_t = cpool.tile([128, NBLK * 16], F32)
            e3 = e_t[:, :].rearrange("p (j k) -> p j k", k=16)
            r3 = r_t[:, :].rearrange("p (j k) -> p j k", k=1)
            r3b = broadcast_tensor_aps(e3, r3)[1]
            nc.vector.tensor_tensor(
                en_t[:, :].rearrange("p (j k) -> p j k", k=16), e3, r3b, op=ALU.mult
            )
            # tmp[p, c, j, k] = en[p, j, k] * C[k, c]  (en broadcast over c)
            tmp_t = cpool.tile([128, 4 * NBLK * 16], F32)
            en4 = en_t[:, :].rearrange("p (c j k) -> p c j k", c=1, k=16)
            cb4 = cbig_t[:, :].rearrange("p (c j k) -> p c j k", c=4, k=16)
            en4b = broadcast_tensor_aps(cb4, en4)[1]
            nc.vector.tensor_tensor(
                tmp_t[:, :].rearrange("p (c j k) -> p c j k", c=4, k=16), en4b, cb4, op=ALU.mult
            )
            w4_t = cpool.tile([128, 4 * NBLK], F32)
            nc.vector.tensor_reduce(
                w4_t[:, :], tmp_t[:, :].rearrange("p (cj k) -> p cj k", k=16), AX.X, op=ALU.add
            )

            def wc(c, j):
                return w4_t[:, c * NBLK + j : c * NBLK + j + 1]

            # ---- block 7 affine prep in early-stream slack (a7 arrives first)
            jl = NBLK - 1
            u7_t = gpool.tile([128, BATCH], F16, tag="u7")
            v7_t = gpool.tile([128, BATCH], F16, tag="v7")
            nc.scalar.activation(
                u7_t[:, :], ga7_t[:, :], AF.Identity, bias=wc(2, jl), scale=wc(3, jl)
            )
            nc.vector.tensor_scalar(
                v7_t[:, :], ga7_t[:, :], wc(1, jl), wc(0, jl), op0=ALU.mult, op1=ALU.add
            )

            o_t = [
                opool.tile([128, BATCH], F16, name=f"o{j}", tag=f"o{j}")
                for j in range(NBLK - 1)
            ]

            # ---- blocks 0..6: streaming compute ----
            for j in range(NBLK - 1):
                a_ap = g_t[j][:, 0:BATCH]
                b_ap = g_t[j][:, BATCH : 2 * BATCH]
                u_t = wpool.tile([128, BATCH], F16, tag="u")
                v_t = wpool.tile([128, BATCH], F16, tag="v")
                t_t = wpool.tile([128, BATCH], F16, tag="t")
                nc.scalar.activation(u_t[:, :], a_ap, AF.Identity, bias=wc(2, j), scale=wc(3, j))
                nc.vector.tensor_scalar(
                    v_t[:, :], a_ap, wc(1, j), wc(0, j), op0=ALU.mult, op1=ALU.add
                )
                nc.vector.tensor_tensor(t_t[:, :], u_t[:, :], b_ap, op=ALU.mult)
                nc.vector.tensor_tensor(o_t[j][:, :], t_t[:, :], v_t[:, :], op=ALU.add)

            # ---- block 7 tail: half-width t+o after b7 (last load) lands ----
            o7_t = [opool.tile([128, HB], F16, name=f"o7{s}", tag=f"o7{s}") for s in range(2)]
            for s in range(2):
                fs = slice(s * HB, (s + 1) * HB)
                t7_t = wpool.tile([128, HB], F16, name=f"t7{s}", tag=f"t7{s}")
                nc.vector.tensor_tensor(t7_t[:, :], u7_t[:, fs], gb7_t[:, fs], op=ALU.mult)
                nc.vector.tensor_tensor(o7_t[s][:, :], t7_t[:, :], v7_t[:, fs], op=ALU.add)

            # ---- stores: SAME sync ring, queued behind all loads (FIFO) ----
            for j in range(NBLK - 1):
                nc.sync.dma_start(ys[j][:, :], o_t[j][:, :])
            for s in range(2):
                nc.sync.dma_start(y7h[s][:, :], o7_t[s][:, :])

    nc.compile()
    return nc


def _get_program():
    global _PROGRAM
    if _PROGRAM is None:
        _PROGRAM = _build_program()
    return _PROGRAM


def make_in_maps(x, weights, indices_a, indices_b):
    x = np.asarray(x, dtype=np.float32)
    w = np.asarray(weights, dtype=np.float32)
    ia = np.asarray(indices_a).astype(np.int64)
    ib = np.asarray(indices_b).astype(np.int64)

    xt16 = np.ascontiguousarray(x.T.astype(np.float16))  # (IN_DIM, BATCH)

    cbig = np.broadcast_to(
        np.tile(_C.T[:, None, :], (1, NBLK, 1)).reshape(1, 4 * NBLK * 16), (128, 4 * NBLK * 16)
    )
    cbig = np.ascontiguousarray(cbig, dtype=np.float32)

    jl = NBLK - 1
    in_maps = []
    for c in range(N_CORES):
        sl = slice(c * OPC, (c + 1) * OPC)
        ia_c = ia[sl].reshape(NBLK, 128)
        ib_c = ib[sl].reshape(NBLK, 128)
        wsh = w[sl]  # (OPC, 16)
        m = {
            "cbig": cbig,
            "wpre": np.ascontiguousarray(
                wsh.reshape(NBLK, 128, 16).transpose(1, 0, 2).reshape(128, NBLK * 16)
            ),
        }
        for j in range(NBLK - 1):
            blk = np.empty((128, 2, BATCH), dtype=np.float16)
            blk[:, 0, :] = xt16[ia_c[j]]
            blk[:, 1, :] = xt16[ib_c[j]]
            m[f"g{j}"] = np.ascontiguousarray(blk.reshape(128, 2 * BATCH))
        m["ga7"] = np.ascontiguousarray(xt16[ia_c[jl]])
        m["gb7"] = np.ascontiguousarray(xt16[ib_c[jl]])
        in_maps.append(m)
    return in_maps


def run(inputs, trace=False):
    if trace:
        try:
            from antenv.axon_hooks import get_axon_ntff_profile_hook  # noqa: F401
        except ImportError:
            trace = False
    nc = _get_program()
    in_maps = make_in_maps(
        inputs["x"], inputs["weights"], inputs["indices_a"], inputs["indices_b"]
    )
    res = run_bass_kernel_spmd(nc, in_maps, core_ids=list(range(N_CORES)), trace=trace)
    outT = np.empty((OUT_DIM, BATCH), dtype=np.float32)
    for c in range(N_CORES):
        r = res.results[c]
        base = c * OPC
        for j in range(NBLK - 1):
            outT[base + j * 128 : base + (j + 1) * 128] = r[f"y{j}"].astype(np.float32)
        o7 = np.concatenate([r["y7h0"], r["y7h1"]], axis=1).astype(np.float32)
        outT[base + 7 * 128 : base + 8 * 128] = o7
    return np.ascontiguousarray(outT.T), res


def kernel(**inputs):
    out, _ = run(inputs, trace=bool(os.environ.get("DL_TRACE")))
    return out


if __name__ == "__main__":
    rng = np.random.default_rng(0)
    inputs = {
        "x": rng.random((BATCH, IN_DIM), dtype=np.float32),
        "weights": rng.standard_normal((OUT_DIM, 16)).astype(np.float32),
        "indices_a": rng.integers(0, IN_DIM, size=OUT_DIM),
        "indices_b": rng.integers(0, IN_DIM, size=OUT_DIM),
    }
    out = kernel(**inputs)
    print(out.shape, out.dtype)


# revision 13
# speedup vs baseline: 1.1629x; 1.0179x over previous
"""DiffLogicLayer Trainium2 kernel (v8: host-gather sharding + fp16 streaming,
phase-serialized DMA).

Math: for each output neuron o with inputs a = x[:, ia[o]], b = x[:, ib[o]],
the 16 relaxed binary gates are all linear in {1, a, b, a*b}:

    gate_k(a, b) = C[k,0] + C[k,1]*a + C[k,2]*b + C[k,3]*a*b

so with w = softmax(weights[o]) the layer output collapses to

    out[n, o] = W0[o] + W1[o]*a + W2[o]*b + W3[o]*a*b,   W = softmax(weights) @ C

Sharding: tensor-parallel over out_dim (1024 neurons/core). The gather
x[:, idx] is pure data movement, so it is folded into the host-side input
sharding: each core receives its 2048 gathered rows of x^T pre-packed fp16.

Measured on this part: HBM reads alone sustain ~440 GB/s, writes ~360,
mixed ~330. So ALL loads and ALL stores go on the SAME sync-HWDGE ring:
FIFO drain => loads stream solo at read bandwidth, stores (queued behind,
gated on compute sems) drain after. ~19 DMAs avoids completion-semaphore
lane cross-gating (v3 lesson). GPSIMD is NOT used for elementwise work: it
shares SBUF ports with DVE and slows concurrent DVE ops ~4x (v5 lesson).
Ops stay full-tile (128, 2048): sliced/strided DVE APs lose the 2x/4x
perf modes (v7 lesson).

Compute per block: u = W3*a + W2 (ACT), v = W1*a + W0 (DVE tensor_scalar,
4x fp16), t = u*b, o = t + v (DVE tensor_tensor, 2x fp16). Softmax+C-fold
fused via stride-0 broadcast APs. Block 7 splits a7 (loaded first, u7/v7
prepped in early slack) from b7 (loaded last; only half-width t+o+store
remain at the tail).

Output fp16; host concatenates, transposes, casts to fp32. Max rel err vs
fp32 reference ~4e-3 (tolerance 2e-2).
"""

import os
import sys

import numpy as np

sys.path.insert(0, "/opt/trn_rl_repo")

import concourse.bacc as bacc
import concourse.mybir as mybir
from concourse import tile
from concourse.bass import broadcast_tensor_aps
from concourse.bass_utils import run_bass_kernel_spmd

AF = mybir.ActivationFunctionType
ALU = mybir.AluOpType
AX = mybir.AxisListType
F32 = mybir.dt.float32
F16 = mybir.dt.float16

IN_DIM = 8192
OUT_DIM = 8192
BATCH = 2048
N_CORES = 8
OPC = OUT_DIM // N_CORES  # 1024 neurons per core
NBLK = OPC // 128  # 8 partition blocks per core
HB = BATCH // 2

# gate_k = C[k,0] + C[k,1]*a + C[k,2]*b + C[k,3]*ab  (difflogic convention)
_C = np.array(
    [
        [0, 0, 0, 0],  # False
        [0, 0, 0, 1],  # a AND b
        [0, 1, 0, -1],  # a AND NOT b
        [0, 1, 0, 0],  # a
        [0, 0, 1, -1],  # NOT a AND b
        [0, 0, 1, 0],  # b
        [0, 1, 1, -2],  # XOR
        [0, 1, 1, -1],  # OR
        [1, -1, -1, 1],  # NOR
        [1, -1, -1, 2],  # XNOR
        [1, 0, -1, 0],  # NOT b
        [1, 0, -1, 1],  # a OR NOT b
        [1, -1, 0, 0],  # NOT a
        [1, -1, 0, 1],  # NOT a OR b
        [1, 0, 0, -1],  # NAND
        [1, 0, 0, 0],  # True
    ],
    dtype=np.float32,
)

_PROGRAM = None


def _build_program():
    nc = bacc.Bacc("TRN2", target_bir_lowering=False, debug=False)

    wpre = nc.dram_tensor("wpre", (128, NBLK * 16), F32, kind="ExternalInput")
    cbig = nc.dram_tensor("cbig", (128, 4 * NBLK * 16), F32, kind="ExternalInput")
    ga7 = nc.dram_tensor("ga7", (128, BATCH), F16, kind="ExternalInput")
    gblk = [
        nc.dram_tensor(f"g{j}", (128, 2 * BATCH), F16, kind="ExternalInput")
        for j in range(NBLK - 1)
    ]
    gb7 = nc.dram_tensor("gb7", (128, BATCH), F16, kind="ExternalInput")
    ys = [
        nc.dram_tensor(f"y{j}", (128, BATCH), F16, kind="ExternalOutput")
        for j in range(NBLK)
    ]

    with tile.TileContext(nc) as tc:
        with (
            tc.tile_pool(name="const", bufs=1) as cpool,
            tc.tile_pool(name="gath", bufs=1) as gpool,
            tc.tile_pool(name="work", bufs=3) as wpool,
            tc.tile_pool(name="outp", bufs=1) as opool,
        ):
            # ---- loads: all on the sync HWDGE ring, in stream order ----
            wpre_t = cpool.tile([128, NBLK * 16], F32)
            nc.sync.dma_start(wpre_t[:, :], wpre[:, :])
            cbig_t = cpool.tile([128, 4 * NBLK * 16], F32)
            nc.sync.dma_start(cbig_t[:, :], cbig[:, :])
            ga7_t = gpool.tile([128, BATCH], F16, tag="ga7")
            nc.sync.dma_start(ga7_t[:, :], ga7[:, :])
            g_t = []
            for j in range(NBLK - 1):
                t = gpool.tile([128, 2 * BATCH], F16, tag=f"g{j}")
                nc.sync.dma_start(t[:, :], gblk[j][:, :])
                g_t.append(t)
            gb7_t = gpool.tile([128, BATCH], F16, tag="gb7")
            nc.sync.dma_start(gb7_t[:, :], gb7[:, :])

            # ---- softmax over the 16 gate logits + C-fold, fused ----
            e_t = cpool.tile([128, NBLK * 16], F32)
            nc.scalar.activation(e_t[:, :], wpre_t[:, :], AF.Exp)
            s_t = cpool.tile([128, NBLK], F32)
            nc.vector.tensor_reduce(
                s_t[:, :], e_t[:, :].rearrange("p (j k) -> p j k", k=16), AX.X, op=ALU.add
            )
            r_t = cpool.tile([128, NBLK], F32)
            nc.vector.reciprocal(r_t[:, :], s_t[:, :])
            # en = softmax = e * (1/s), with 1/s broadcast over k (stride-0)
            en_t = cpool.tile([128, NBLK * 16], F32)
            e3 = e_t[:, :].rearrange("p (j k) -> p j k", k=16)
            r3 = r_t[:, :].rearrange("p (j k) -> p j k", k=1)
            r3b = broadcast_tensor_aps(e3, r3)[1]
            nc.vector.tensor_tensor(
                en_t[:, :].rearrange("p (j k) -> p j k", k=16), e3, r3b, op=ALU.mult
            )
            # tmp[p, c, j, k] = en[p, j, k] * C[k, c]  (en broadcast over c)
            tmp_t = cpool.tile([128, 4 * NBLK * 16], F32)
            en4 = en_t[:, :].rearrange("p (c j k) -> p c j k", c=1, k=16)
            cb4 = cbig_t[:, :].rearrange("p (c j k) -> p c j k", c=4, k=16)
            en4b = broadcast_tensor_aps(cb4, en4)[1]
            nc.vector.tensor_tensor(
                tmp_t[:, :].rearrange("p (c j k) -> p c j k", c=4, k=16), en4b, cb4, op=ALU.mult
            )
            w4_t = cpool.tile([128, 4 * NBLK], F32)
            nc.vector.tensor_reduce(
                w4_t[:, :], tmp_t[:, :].rearrange("p (cj k) -> p cj k", k=16), AX.X, op=ALU.add
            )

            def wc(c, j):
                return w4_t[:, c * NBLK + j : c * NBLK + j + 1]

            # ---- block 7 affine prep in early-stream slack (a7 arrives first)
            jl = NBLK - 1
            u7_t = gpool.tile([128, BATCH], F16, tag="u7")
            v7_t = gpool.tile([128, BATCH], F16, tag="v7")
            nc.scalar.activation(
                u7_t[:, :], ga7_t[:, :], AF.Identity, bias=wc(2, jl), scale=wc(3, jl)
            )
            nc.vector.tensor_scalar(
                v7_t[:, :], ga7_t[:, :], wc(1, jl), wc(0, jl), op0=ALU.mult, op1=ALU.add
            )

            o_t = [
                opool.tile([128, BATCH], F16, name=f"o{j}", tag=f"o{j}")
                for j in range(NBLK)
            ]

            # ---- blocks 0..6: streaming compute ----
            for j in range(NBLK - 1):
                a_ap = g_t[j][:, 0:BATCH]
                b_ap = g_t[j][:, BATCH : 2 * BATCH]
                u_t = wpool.tile([128, BATCH], F16, tag="u")
                v_t = wpool.tile([128, BATCH], F16, tag="v")
                t_t = wpool.tile([128, BATCH], F16, tag="t")
                nc.scalar.activation(u_t[:, :], a_ap, AF.Identity, bias=wc(2, j), scale=wc(3, j))
                nc.vector.tensor_scalar(
                    v_t[:, :], a_ap, wc(1, j), wc(0, j), op0=ALU.mult, op1=ALU.add
                )
                nc.vector.tensor_tensor(t_t[:, :], u_t[:, :], b_ap, op=ALU.mult)
                nc.vector.tensor_tensor(o_t[j][:, :], t_t[:, :], v_t[:, :], op=ALU.add)

            # ---- block 7 tail: only t+o remain after b7 (last load) lands ----
            t7_t = wpool.tile([128, BATCH], F16, tag="t7")
            nc.vector.tensor_tensor(t7_t[:, :], u7_t[:, :], gb7_t[:, :], op=ALU.mult)
            nc.vector.tensor_tensor(o_t[jl][:, :], t7_t[:, :], v7_t[:, :], op=ALU.add)

            # ---- stores: SAME sync ring, queued behind all loads (FIFO) ----
            for j in range(NBLK):
                nc.sync.dma_start(ys[j][:, :], o_t[j][:, :])

    nc.compile()
    return nc


def _get_program():
    global _PROGRAM
    if _PROGRAM is None:
        _PROGRAM = _build_program()
    return _PROGRAM


def make_in_maps(x, weights, indices_a, indices_b):
    x = np.asarray(x, dtype=np.float32)
    w = np.asarray(weights, dtype=np.float32)
    ia = np.asarray(indices_a).astype(np.int64)
    ib = np.asarray(indices_b).astype(np.int64)

    xt16 = np.ascontiguousarray(x.T.astype(np.float16))  # (IN_DIM, BATCH)

    cbig = np.broadcast_to(
        np.tile(_C.T[:, None, :], (1, NBLK, 1)).reshape(1, 4 * NBLK * 16), (128, 4 * NBLK * 16)
    )
    cbig = np.ascontiguousarray(cbig, dtype=np.float32)

    jl = NBLK - 1
    in_maps = []
    for c in range(N_CORES):
        sl = slice(c * OPC, (c + 1) * OPC)
        ia_c = ia[sl].reshape(NBLK, 128)
        ib_c = ib[sl].reshape(NBLK, 128)
        wsh = w[sl]  # (OPC, 16)
        m = {
            "cbig": cbig,
            "wpre": np.ascontiguousarray(
                wsh.reshape(NBLK, 128, 16).transpose(1, 0, 2).reshape(128, NBLK * 16)
            ),
        }
        for j in range(NBLK - 1):
            blk = np.empty((128, 2, BATCH), dtype=np.float16)
            blk[:, 0, :] = xt16[ia_c[j]]
            blk[:, 1, :] = xt16[ib_c[j]]
            m[f"g{j}"] = np.ascontiguousarray(blk.reshape(128, 2 * BATCH))
        m["ga7"] = np.ascontiguousarray(xt16[ia_c[jl]])
        m["gb7"] = np.ascontiguousarray(xt16[ib_c[jl]])
        in_maps.append(m)
    return in_maps


def run(inputs, trace=False):
    if trace:
        try:
            from antenv.axon_hooks import get_axon_ntff_profile_hook  # noqa: F401
        except ImportError:
            trace = False
    nc = _get_program()
    in_maps = make_in_maps(
        inputs["x"], inputs["weights"], inputs["indices_a"], inputs["indices_b"]
    )
    res = run_bass_kernel_spmd(nc, in_maps, core_ids=list(range(N_CORES)), trace=trace)
    outT = np.empty((OUT_DIM, BATCH), dtype=np.float32)
    for c in range(N_CORES):
        r = res.results[c]
        base = c * OPC
        for j in range(NBLK):
            outT[base + j * 128 : base + (j + 1) * 128] = r[f"y{j}"].astype(np.float32)
    return np.ascontiguousarray(outT.T), res


def kernel(**inputs):
    out, _ = run(inputs, trace=bool(os.environ.get("DL_TRACE")))
    return out


if __name__ == "__main__":
    rng = np.random.default_rng(0)
    inputs = {
        "x": rng.random((BATCH, IN_DIM), dtype=np.float32),
        "weights": rng.standard_normal((OUT_DIM, 16)).astype(np.float32),
        "indices_a": rng.integers(0, IN_DIM, size=OUT_DIM),
        "indices_b": rng.integers(0, IN_DIM, size=OUT_DIM),
    }
    out = kernel(**inputs)
    print(out.shape, out.dtype)


# revision 14
# speedup vs baseline: 1.2092x; 1.0399x over previous
"""DiffLogicLayer Trainium2 kernel (v8: host-gather sharding + fp16 streaming,
phase-serialized DMA).

Math: for each output neuron o with inputs a = x[:, ia[o]], b = x[:, ib[o]],
the 16 relaxed binary gates are all linear in {1, a, b, a*b}:

    gate_k(a, b) = C[k,0] + C[k,1]*a + C[k,2]*b + C[k,3]*a*b

so with w = softmax(weights[o]) the layer output collapses to

    out[n, o] = W0[o] + W1[o]*a + W2[o]*b + W3[o]*a*b,   W = softmax(weights) @ C

Sharding: tensor-parallel over out_dim (1024 neurons/core). The gather
x[:, idx] is pure data movement, so it is folded into the host-side input
sharding: each core receives its 2048 gathered rows of x^T pre-packed fp16.

Measured on this part: HBM reads alone sustain ~440 GB/s, writes ~360,
mixed ~330. So ALL loads and ALL stores go on the SAME sync-HWDGE ring:
FIFO drain => loads stream solo at read bandwidth, stores (queued behind,
gated on compute sems) drain after. ~19 DMAs avoids completion-semaphore
lane cross-gating (v3 lesson). GPSIMD is NOT used for elementwise work: it
shares SBUF ports with DVE and slows concurrent DVE ops ~4x (v5 lesson).
Ops stay full-tile (128, 2048): sliced/strided DVE APs lose the 2x/4x
perf modes (v7 lesson).

Compute per block: u = W3*a + W2 (ACT), v = W1*a + W0 (DVE tensor_scalar,
4x fp16), t = u*b, o = t + v (DVE tensor_tensor, 2x fp16). Softmax+C-fold
fused via stride-0 broadcast APs. Block 7 splits a7 (loaded first, u7/v7
prepped in early slack) from b7 (loaded last; only half-width t+o+store
remain at the tail).

Output fp16; host concatenates, transposes, casts to fp32. Max rel err vs
fp32 reference ~4e-3 (tolerance 2e-2).
"""

import os
import sys

import numpy as np

sys.path.insert(0, "/opt/trn_rl_repo")

import concourse.bacc as bacc
import concourse.mybir as mybir
from concourse import tile
from concourse.bass import broadcast_tensor_aps
from concourse.bass_utils import run_bass_kernel_spmd

AF = mybir.ActivationFunctionType
ALU = mybir.AluOpType
AX = mybir.AxisListType
F32 = mybir.dt.float32
F16 = mybir.dt.float16

IN_DIM = 8192
OUT_DIM = 8192
BATCH = 2048
N_CORES = 8
OPC = OUT_DIM // N_CORES  # 1024 neurons per core
NBLK = OPC // 128  # 8 partition blocks per core
HB = BATCH // 2

# gate_k = C[k,0] + C[k,1]*a + C[k,2]*b + C[k,3]*ab  (difflogic convention)
_C = np.array(
    [
        [0, 0, 0, 0],  # False
        [0, 0, 0, 1],  # a AND b
        [0, 1, 0, -1],  # a AND NOT b
        [0, 1, 0, 0],  # a
        [0, 0, 1, -1],  # NOT a AND b
        [0, 0, 1, 0],  # b
        [0, 1, 1, -2],  # XOR
        [0, 1, 1, -1],  # OR
        [1, -1, -1, 1],  # NOR
        [1, -1, -1, 2],  # XNOR
        [1, 0, -1, 0],  # NOT b
        [1, 0, -1, 1],  # a OR NOT b
        [1, -1, 0, 0],  # NOT a
        [1, -1, 0, 1],  # NOT a OR b
        [1, 0, 0, -1],  # NAND
        [1, 0, 0, 0],  # True
    ],
    dtype=np.float32,
)

_PROGRAM = None


def _build_program():
    nc = bacc.Bacc("TRN2", target_bir_lowering=False, debug=False)

    wpre = nc.dram_tensor("wpre", (128, NBLK * 16), F32, kind="ExternalInput")
    cbig = nc.dram_tensor("cbig", (128, 4 * NBLK * 16), F32, kind="ExternalInput")
    ga7 = nc.dram_tensor("ga7", (128, BATCH), F16, kind="ExternalInput")
    gblk = [
        nc.dram_tensor(f"g{j}", (128, 2 * BATCH), F16, kind="ExternalInput")
        for j in range(NBLK - 1)
    ]
    gb7 = nc.dram_tensor("gb7", (128, BATCH), F16, kind="ExternalInput")
    ys = [
        nc.dram_tensor(f"y{j}", (128, BATCH), F16, kind="ExternalOutput")
        for j in range(NBLK)
    ]

    with tile.TileContext(nc) as tc:
        with (
            tc.tile_pool(name="const", bufs=1) as cpool,
            tc.tile_pool(name="gath", bufs=1) as gpool,
            tc.tile_pool(name="work", bufs=3) as wpool,
            tc.tile_pool(name="outp", bufs=1) as opool,
        ):
            # ---- loads: all on the sync HWDGE ring, in stream order ----
            wpre_t = cpool.tile([128, NBLK * 16], F32)
            nc.sync.dma_start(wpre_t[:, :], wpre[:, :])
            cbig_t = cpool.tile([128, 4 * NBLK * 16], F32)
            nc.sync.dma_start(cbig_t[:, :], cbig[:, :])
            g_t = []
            for j in range(NBLK - 1):
                t = gpool.tile([128, 2 * BATCH], F16, tag=f"g{j}")
                nc.sync.dma_start(t[:, :], gblk[j][:, :])
                g_t.append(t)
                if j == 0:
                    # a7 loads after g0: block 0 feeds the engines first, and
                    # u7/v7 prep still finishes long before b7 (last) lands.
                    ga7_t = gpool.tile([128, BATCH], F16, tag="ga7")
                    nc.sync.dma_start(ga7_t[:, :], ga7[:, :])
            gb7_t = gpool.tile([128, BATCH], F16, tag="gb7")
            nc.sync.dma_start(gb7_t[:, :], gb7[:, :])

            # ---- softmax over the 16 gate logits + C-fold, fused ----
            e_t = cpool.tile([128, NBLK * 16], F32)
            nc.scalar.activation(e_t[:, :], wpre_t[:, :], AF.Exp)
            s_t = cpool.tile([128, NBLK], F32)
            nc.vector.tensor_reduce(
                s_t[:, :], e_t[:, :].rearrange("p (j k) -> p j k", k=16), AX.X, op=ALU.add
            )
            r_t = cpool.tile([128, NBLK], F32)
            nc.vector.reciprocal(r_t[:, :], s_t[:, :])
            # en = softmax = e * (1/s), with 1/s broadcast over k (stride-0)
            en_t = cpool.tile([128, NBLK * 16], F32)
            e3 = e_t[:, :].rearrange("p (j k) -> p j k", k=16)
            r3 = r_t[:, :].rearrange("p (j k) -> p j k", k=1)
            r3b = broadcast_tensor_aps(e3, r3)[1]
            nc.vector.tensor_tensor(
                en_t[:, :].rearrange("p (j k) -> p j k", k=16), e3, r3b, op=ALU.mult
            )
            # tmp[p, c, j, k] = en[p, j, k] * C[k, c]  (en broadcast over c)
            tmp_t = cpool.tile([128, 4 * NBLK * 16], F32)
            en4 = en_t[:, :].rearrange("p (c j k) -> p c j k", c=1, k=16)
            cb4 = cbig_t[:, :].rearrange("p (c j k) -> p c j k", c=4, k=16)
            en4b = broadcast_tensor_aps(cb4, en4)[1]
            nc.vector.tensor_tensor(
                tmp_t[:, :].rearrange("p (c j k) -> p c j k", c=4, k=16), en4b, cb4, op=ALU.mult
            )
            w4_t = cpool.tile([128, 4 * NBLK], F32)
            nc.vector.tensor_reduce(
                w4_t[:, :], tmp_t[:, :].rearrange("p (cj k) -> p cj k", k=16), AX.X, op=ALU.add
            )

            def wc(c, j):
                return w4_t[:, c * NBLK + j : c * NBLK + j + 1]

            jl = NBLK - 1
            u7_t = gpool.tile([128, BATCH], F16, tag="u7")
            v7_t = gpool.tile([128, BATCH], F16, tag="v7")

            o_t = [
                opool.tile([128, BATCH], F16, name=f"o{j}", tag=f"o{j}")
                for j in range(NBLK)
            ]

            # ---- blocks 0..6: streaming compute ----
            for j in range(NBLK - 1):
                a_ap = g_t[j][:, 0:BATCH]
                b_ap = g_t[j][:, BATCH : 2 * BATCH]
                u_t = wpool.tile([128, BATCH], F16, tag="u")
                v_t = wpool.tile([128, BATCH], F16, tag="v")
                t_t = wpool.tile([128, BATCH], F16, tag="t")
                nc.scalar.activation(u_t[:, :], a_ap, AF.Identity, bias=wc(2, j), scale=wc(3, j))
                nc.vector.tensor_scalar(
                    v_t[:, :], a_ap, wc(1, j), wc(0, j), op0=ALU.mult, op1=ALU.add
                )
                nc.vector.tensor_tensor(t_t[:, :], u_t[:, :], b_ap, op=ALU.mult)
                nc.vector.tensor_tensor(o_t[j][:, :], t_t[:, :], v_t[:, :], op=ALU.add)
                if j == 1:
                    # block 7 affine prep in mid-stream slack (a7 landed by now)
                    nc.scalar.activation(
                        u7_t[:, :], ga7_t[:, :], AF.Identity, bias=wc(2, jl), scale=wc(3, jl)
                    )
                    nc.vector.tensor_scalar(
                        v7_t[:, :], ga7_t[:, :], wc(1, jl), wc(0, jl), op0=ALU.mult, op1=ALU.add
                    )

            # ---- block 7 tail: only t+o remain after b7 (last load) lands ----
            t7_t = wpool.tile([128, BATCH], F16, tag="t7")
            nc.vector.tensor_tensor(t7_t[:, :], u7_t[:, :], gb7_t[:, :], op=ALU.mult)
            nc.vector.tensor_tensor(o_t[jl][:, :], t7_t[:, :], v7_t[:, :], op=ALU.add)

            # ---- stores: SAME sync ring, queued behind all loads (FIFO) ----
            for j in range(NBLK):
                nc.sync.dma_start(ys[j][:, :], o_t[j][:, :])

    nc.compile()
    return nc


def _get_program():
    global _PROGRAM
    if _PROGRAM is None:
        _PROGRAM = _build_program()
    return _PROGRAM


def make_in_maps(x, weights, indices_a, indices_b):
    x = np.asarray(x, dtype=np.float32)
    w = np.asarray(weights, dtype=np.float32)
    ia = np.asarray(indices_a).astype(np.int64)
    ib = np.asarray(indices_b).astype(np.int64)

    xt16 = np.ascontiguousarray(x.T.astype(np.float16))  # (IN_DIM, BATCH)

    cbig = np.broadcast_to(
        np.tile(_C.T[:, None, :], (1, NBLK, 1)).reshape(1, 4 * NBLK * 16), (128, 4 * NBLK * 16)
    )
    cbig = np.ascontiguousarray(cbig, dtype=np.float32)

    jl = NBLK - 1
    in_maps = []
    for c in range(N_CORES):
        sl = slice(c * OPC, (c + 1) * OPC)
        ia_c = ia[sl].reshape(NBLK, 128)
        ib_c = ib[sl].reshape(NBLK, 128)
        wsh = w[sl]  # (OPC, 16)
        m = {
            "cbig": cbig,
            "wpre": np.ascontiguousarray(
                wsh.reshape(NBLK, 128, 16).transpose(1, 0, 2).reshape(128, NBLK * 16)
            ),
        }
        for j in range(NBLK - 1):
            blk = np.empty((128, 2, BATCH), dtype=np.float16)
            blk[:, 0, :] = xt16[ia_c[j]]
            blk[:, 1, :] = xt16[ib_c[j]]
            m[f"g{j}"] = np.ascontiguousarray(blk.reshape(128, 2 * BATCH))
        m["ga7"] = np.ascontiguousarray(xt16[ia_c[jl]])
        m["gb7"] = np.ascontiguousarray(xt16[ib_c[jl]])
        in_maps.append(m)
    return in_maps


def run(inputs, trace=False):
    if trace:
        try:
            from antenv.axon_hooks import get_axon_ntff_profile_hook  # noqa: F401
        except ImportError:
            trace = False
    nc = _get_program()
    in_maps = make_in_maps(
        inputs["x"], inputs["weights"], inputs["indices_a"], inputs["indices_b"]
    )
    res = run_bass_kernel_spmd(nc, in_maps, core_ids=list(range(N_CORES)), trace=trace)
    outT = np.empty((OUT_DIM, BATCH), dtype=np.float32)
    for c in range(N_CORES):
        r = res.results[c]
        base = c * OPC
        for j in range(NBLK):
            outT[base + j * 128 : base + (j + 1) * 128] = r[f"y{j}"].astype(np.float32)
    return np.ascontiguousarray(outT.T), res


def kernel(**inputs):
    out, _ = run(inputs, trace=bool(os.environ.get("DL_TRACE")))
    return out


if __name__ == "__main__":
    rng = np.random.default_rng(0)
    inputs = {
        "x": rng.random((BATCH, IN_DIM), dtype=np.float32),
        "weights": rng.standard_normal((OUT_DIM, 16)).astype(np.float32),
        "indices_a": rng.integers(0, IN_DIM, size=OUT_DIM),
        "indices_b": rng.integers(0, IN_DIM, size=OUT_DIM),
    }
    out = kernel(**inputs)
    print(out.shape, out.dtype)


# revision 15
# speedup vs baseline: 1.2179x; 1.0071x over previous
"""DiffLogicLayer Trainium2 kernel (v8: host-gather sharding + fp16 streaming,
phase-serialized DMA).

Math: for each output neuron o with inputs a = x[:, ia[o]], b = x[:, ib[o]],
the 16 relaxed binary gates are all linear in {1, a, b, a*b}:

    gate_k(a, b) = C[k,0] + C[k,1]*a + C[k,2]*b + C[k,3]*a*b

so with w = softmax(weights[o]) the layer output collapses to

    out[n, o] = W0[o] + W1[o]*a + W2[o]*b + W3[o]*a*b,   W = softmax(weights) @ C

Sharding: tensor-parallel over out_dim (1024 neurons/core). The gather
x[:, idx] is pure data movement, so it is folded into the host-side input
sharding: each core receives its 2048 gathered rows of x^T pre-packed fp16.

Measured on this part: HBM reads alone sustain ~440 GB/s, writes ~360,
mixed ~330. So ALL loads and ALL stores go on the SAME sync-HWDGE ring:
FIFO drain => loads stream solo at read bandwidth, stores (queued behind,
gated on compute sems) drain after. ~19 DMAs avoids completion-semaphore
lane cross-gating (v3 lesson). GPSIMD is NOT used for elementwise work: it
shares SBUF ports with DVE and slows concurrent DVE ops ~4x (v5 lesson).
Ops stay full-tile (128, 2048): sliced/strided DVE APs lose the 2x/4x
perf modes (v7 lesson).

Compute per block: u = W3*a + W2 (ACT), v = W1*a + W0 (DVE tensor_scalar,
4x fp16), t = u*b, o = t + v (DVE tensor_tensor, 2x fp16). Softmax+C-fold
fused via stride-0 broadcast APs. Block 7 splits a7 (loaded first, u7/v7
prepped in early slack) from b7 (loaded last; only half-width t+o+store
remain at the tail).

Output fp16; host concatenates, transposes, casts to fp32. Max rel err vs
fp32 reference ~4e-3 (tolerance 2e-2).
"""

import os
import sys

import numpy as np

sys.path.insert(0, "/opt/trn_rl_repo")

import concourse.bacc as bacc
import concourse.mybir as mybir
from concourse import tile
from concourse.bass import broadcast_tensor_aps
from concourse.bass_utils import run_bass_kernel_spmd

AF = mybir.ActivationFunctionType
ALU = mybir.AluOpType
AX = mybir.AxisListType
F32 = mybir.dt.float32
F16 = mybir.dt.float16

IN_DIM = 8192
OUT_DIM = 8192
BATCH = 2048
N_CORES = 8
OPC = OUT_DIM // N_CORES  # 1024 neurons per core
NBLK = OPC // 128  # 8 partition blocks per core
HB = BATCH // 2

# gate_k = C[k,0] + C[k,1]*a + C[k,2]*b + C[k,3]*ab  (difflogic convention)
_C = np.array(
    [
        [0, 0, 0, 0],  # False
        [0, 0, 0, 1],  # a AND b
        [0, 1, 0, -1],  # a AND NOT b
        [0, 1, 0, 0],  # a
        [0, 0, 1, -1],  # NOT a AND b
        [0, 0, 1, 0],  # b
        [0, 1, 1, -2],  # XOR
        [0, 1, 1, -1],  # OR
        [1, -1, -1, 1],  # NOR
        [1, -1, -1, 2],  # XNOR
        [1, 0, -1, 0],  # NOT b
        [1, 0, -1, 1],  # a OR NOT b
        [1, -1, 0, 0],  # NOT a
        [1, -1, 0, 1],  # NOT a OR b
        [1, 0, 0, -1],  # NAND
        [1, 0, 0, 0],  # True
    ],
    dtype=np.float32,
)

_PROGRAM = None


def _build_program():
    nc = bacc.Bacc("TRN2", target_bir_lowering=False, debug=False)

    wpre = nc.dram_tensor("wpre", (128, NBLK * 16), F32, kind="ExternalInput")
    cb64 = nc.dram_tensor("cb64", (128, 4 * 16), F32, kind="ExternalInput")
    ga7 = nc.dram_tensor("ga7", (128, BATCH), F16, kind="ExternalInput")
    gblk = [
        nc.dram_tensor(f"g{j}", (128, 2 * BATCH), F16, kind="ExternalInput")
        for j in range(NBLK - 1)
    ]
    gb7 = nc.dram_tensor("gb7", (128, BATCH), F16, kind="ExternalInput")
    ys = [
        nc.dram_tensor(f"y{j}", (128, BATCH), F16, kind="ExternalOutput")
        for j in range(NBLK)
    ]

    with tile.TileContext(nc) as tc:
        with (
            tc.tile_pool(name="const", bufs=1) as cpool,
            tc.tile_pool(name="gath", bufs=1) as gpool,
            tc.tile_pool(name="work", bufs=3) as wpool,
            tc.tile_pool(name="outp", bufs=1) as opool,
        ):
            # ---- loads: all on the sync HWDGE ring, in stream order ----
            wpre_t = cpool.tile([128, NBLK * 16], F32)
            nc.sync.dma_start(wpre_t[:, :], wpre[:, :])
            cb64_t = cpool.tile([128, 4 * 16], F32)
            nc.sync.dma_start(cb64_t[:, :], cb64[:, :])
            g_t = []
            for j in range(NBLK - 1):
                t = gpool.tile([128, 2 * BATCH], F16, tag=f"g{j}")
                nc.sync.dma_start(t[:, :], gblk[j][:, :])
                g_t.append(t)
                if j == 0:
                    # a7 loads after g0: block 0 feeds the engines first, and
                    # u7/v7 prep still finishes long before b7 (last) lands.
                    ga7_t = gpool.tile([128, BATCH], F16, tag="ga7")
                    nc.sync.dma_start(ga7_t[:, :], ga7[:, :])
            gb7_t = gpool.tile([128, BATCH], F16, tag="gb7")
            nc.sync.dma_start(gb7_t[:, :], gb7[:, :])

            # ---- softmax over the 16 gate logits + C-fold, fused ----
            e_t = cpool.tile([128, NBLK * 16], F32)
            nc.scalar.activation(e_t[:, :], wpre_t[:, :], AF.Exp)
            s_t = cpool.tile([128, NBLK], F32)
            nc.vector.tensor_reduce(
                s_t[:, :], e_t[:, :].rearrange("p (j k) -> p j k", k=16), AX.X, op=ALU.add
            )
            r_t = cpool.tile([128, NBLK], F32)
            nc.vector.reciprocal(r_t[:, :], s_t[:, :])
            # tmp[p, c, j, k] = e[p, j, k] * C[k, c]  (e bcast over c, C over j)
            tmp_t = cpool.tile([128, 4 * NBLK * 16], F32)
            tmp4 = tmp_t[:, :].rearrange("p (c j k) -> p c j k", c=4, k=16)
            e4 = e_t[:, :].rearrange("p (c j k) -> p c j k", c=1, k=16)
            e4b = broadcast_tensor_aps(tmp4, e4)[1]
            cbj = cb64_t[:, :].rearrange("p (c j k) -> p c j k", c=4, k=16)
            cbjb = broadcast_tensor_aps(tmp4, cbj)[1]
            nc.vector.tensor_tensor(tmp4, e4b, cbjb, op=ALU.mult)
            raw_t = cpool.tile([128, 4 * NBLK], F32)
            nc.vector.tensor_reduce(
                raw_t[:, :], tmp_t[:, :].rearrange("p (cj k) -> p cj k", k=16), AX.X, op=ALU.add
            )
            # w4 = raw * (1/s), with 1/s broadcast over c (stride-0)
            w4_t = cpool.tile([128, 4 * NBLK], F32)
            w43 = w4_t[:, :].rearrange("p (c j) -> p c j", c=4)
            r43 = r_t[:, :].rearrange("p (c j) -> p c j", c=1)
            r43b = broadcast_tensor_aps(w43, r43)[1]
            nc.vector.tensor_tensor(
                w43, raw_t[:, :].rearrange("p (c j) -> p c j", c=4), r43b, op=ALU.mult
            )

            def wc(c, j):
                return w4_t[:, c * NBLK + j : c * NBLK + j + 1]

            jl = NBLK - 1
            u7_t = gpool.tile([128, BATCH], F16, tag="u7")
            v7_t = gpool.tile([128, BATCH], F16, tag="v7")

            o_t = [
                opool.tile([128, BATCH], F16, name=f"o{j}", tag=f"o{j}")
                for j in range(NBLK)
            ]

            # ---- blocks 0..6: streaming compute ----
            for j in range(NBLK - 1):
                a_ap = g_t[j][:, 0:BATCH]
                b_ap = g_t[j][:, BATCH : 2 * BATCH]
                u_t = wpool.tile([128, BATCH], F16, tag="u")
                v_t = wpool.tile([128, BATCH], F16, tag="v")
                t_t = wpool.tile([128, BATCH], F16, tag="t")
                nc.scalar.activation(u_t[:, :], a_ap, AF.Identity, bias=wc(2, j), scale=wc(3, j))
                nc.vector.tensor_scalar(
                    v_t[:, :], a_ap, wc(1, j), wc(0, j), op0=ALU.mult, op1=ALU.add
                )
                nc.vector.tensor_tensor(t_t[:, :], u_t[:, :], b_ap, op=ALU.mult)
                nc.vector.tensor_tensor(o_t[j][:, :], t_t[:, :], v_t[:, :], op=ALU.add)
                if j == 1:
                    # block 7 affine prep in mid-stream slack (a7 landed by now)
                    nc.scalar.activation(
                        u7_t[:, :], ga7_t[:, :], AF.Identity, bias=wc(2, jl), scale=wc(3, jl)
                    )
                    nc.vector.tensor_scalar(
                        v7_t[:, :], ga7_t[:, :], wc(1, jl), wc(0, jl), op0=ALU.mult, op1=ALU.add
                    )

            # ---- block 7 tail: only t+o remain after b7 (last load) lands ----
            t7_t = wpool.tile([128, BATCH], F16, tag="t7")
            nc.vector.tensor_tensor(t7_t[:, :], u7_t[:, :], gb7_t[:, :], op=ALU.mult)
            nc.vector.tensor_tensor(o_t[jl][:, :], t7_t[:, :], v7_t[:, :], op=ALU.add)

            # ---- stores: SAME sync ring, queued behind all loads (FIFO) ----
            for j in range(NBLK):
                nc.sync.dma_start(ys[j][:, :], o_t[j][:, :])

    nc.compile()
    return nc


def _get_program():
    global _PROGRAM
    if _PROGRAM is None:
        _PROGRAM = _build_program()
    return _PROGRAM


def make_in_maps(x, weights, indices_a, indices_b):
    x = np.asarray(x, dtype=np.float32)
    w = np.asarray(weights, dtype=np.float32)
    ia = np.asarray(indices_a).astype(np.int64)
    ib = np.asarray(indices_b).astype(np.int64)

    xt16 = np.ascontiguousarray(x.T.astype(np.float16))  # (IN_DIM, BATCH)

    cb64 = np.ascontiguousarray(
        np.broadcast_to(_C.T.reshape(1, 64), (128, 64)), dtype=np.float32
    )

    jl = NBLK - 1
    in_maps = []
    for c in range(N_CORES):
        sl = slice(c * OPC, (c + 1) * OPC)
        ia_c = ia[sl].reshape(NBLK, 128)
        ib_c = ib[sl].reshape(NBLK, 128)
        wsh = w[sl]  # (OPC, 16)
        m = {
            "cb64": cb64,
            "wpre": np.ascontiguousarray(
                wsh.reshape(NBLK, 128, 16).transpose(1, 0, 2).reshape(128, NBLK * 16)
            ),
        }
        for j in range(NBLK - 1):
            blk = np.empty((128, 2, BATCH), dtype=np.float16)
            blk[:, 0, :] = xt16[ia_c[j]]
            blk[:, 1, :] = xt16[ib_c[j]]
            m[f"g{j}"] = np.ascontiguousarray(blk.reshape(128, 2 * BATCH))
        m["ga7"] = np.ascontiguousarray(xt16[ia_c[jl]])
        m["gb7"] = np.ascontiguousarray(xt16[ib_c[jl]])
        in_maps.append(m)
    return in_maps


def run(inputs, trace=False):
    if trace:
        try:
            from antenv.axon_hooks import get_axon_ntff_profile_hook  # noqa: F401
        except ImportError:
            trace = False
    nc = _get_program()
    in_maps = make_in_maps(
        inputs["x"], inputs["weights"], inputs["indices_a"], inputs["indices_b"]
    )
    res = run_bass_kernel_spmd(nc, in_maps, core_ids=list(range(N_CORES)), trace=trace)
    outT = np.empty((OUT_DIM, BATCH), dtype=np.float32)
    for c in range(N_CORES):
        r = res.results[c]
        base = c * OPC
        for j in range(NBLK):
            outT[base + j * 128 : base + (j + 1) * 128] = r[f"y{j}"].astype(np.float32)
    return np.ascontiguousarray(outT.T), res


def kernel(**inputs):
    out, _ = run(inputs, trace=bool(os.environ.get("DL_TRACE")))
    return out


if __name__ == "__main__":
    rng = np.random.default_rng(0)
    inputs = {
        "x": rng.random((BATCH, IN_DIM), dtype=np.float32),
        "weights": rng.standard_normal((OUT_DIM, 16)).astype(np.float32),
        "indices_a": rng.integers(0, IN_DIM, size=OUT_DIM),
        "indices_b": rng.integers(0, IN_DIM, size=OUT_DIM),
    }
    out = kernel(**inputs)
    print(out.shape, out.dtype)
